# revision 1
# baseline (speedup 1.0000x reference)
"""BiLSTM-CRF NLL kernel for 8 Trainium2 NeuronCores.

Strategy (3 SPMD launches, host glue between them):
  L1 "layer0": 8 cores = 2 dirs x 4 batch-quarters (16 seqs/core, one LSTM dir).
     Per core: gx = W_ih @ x^T (+biases) as chunked matmuls interleaved with the
     256-step recurrent scan (weight-stationary matmuls; gates in a
     [128-partition, batch-free] layout so elementwise ops use all lanes).
  L2 "layer1": same program shape with K=512 input; host reshards and handles
     the per-sequence reversal of the backward direction.
  L3 "logits+CRF": 8 cores = 8 batch-eighths. Logits matmul, then the CRF
     partition function as an exp-domain matrix recursion
     a_t = (E^T a_{t-1}) * exp(logit_t), E = exp(trans) on the PE, periodic
     per-seq renormalization. Masking is avoided by keeping the whole a_t
     history and extracting column t=len_b-1 per sequence via a host-built
     one-hot mask. start/end/transition numerator terms are summed on host.

Matmuls run in bf16 (fp32 PSUM accumulate); cell state c and CRF are fp32.
"""

import os
import sys

import numpy as np

for _p in ("/opt/trn_rl_repo", "/root/.axon_site/_ro/trn_rl_repo"):
    if _p not in sys.path and os.path.isdir(_p):
        sys.path.insert(0, _p)

import ml_dtypes  # noqa: E402

BF16 = ml_dtypes.bfloat16

B, T, V, E, HD, NT = 64, 256, 50000, 256, 256, 20
NCORES = 8
BL = 16            # sequences per core in L1/L2 (batch quarter)
BC = 8             # sequences per core in L3 (batch eighth)
NTOK = BL * T      # tokens per core in L1/L2
NTOK3 = BC * T     # tokens per core in L3
NJ = 8             # gate tiles of 128 rows (4 gates x 256 HD / 128)
NCH = 512          # matmul N-chunk (tokens)
TCH = NCH // BL    # timesteps per gx chunk (32)
RENORM_EVERY = 8   # CRF renormalization interval
NREN = (T - 1) // RENORM_EVERY   # renorm slots used (t = 8,16,...,248)

# gate order stays pytorch-native (i,f,g,o): the c-path gates (i,f,g) are
# tiles 0..5 (one contiguous sigmoid), o is tiles 6..7 (deferred off the
# critical path). g rows are pre-scaled by 2 so tanh(x) = 2*sig(2x)-1.
_PERM = np.arange(4 * HD)

_CACHE = {}
LAST_RESULTS = []   # BassKernelResults of the launches of the last kernel() call


def _mods():
    import concourse.bass as bass
    import concourse.tile as tile
    from concourse import bacc, mybir
    from concourse.bass_utils import run_bass_kernel_spmd
    return bass, tile, bacc, mybir, run_bass_kernel_spmd


def _install_ntff_shim():
    """Provide antenv.axon_hooks (missing in this image) so that
    run_bass_kernel_spmd(trace=True) can capture NTFF profiles through
    libaxon_pjrt.so. Mirrors trn_agent_boot._ntff_profile_via_ctypes."""
    import sys as _sys
    if "antenv.axon_hooks" in _sys.modules:
        return
    import contextlib
    import ctypes
    import types

    so_path = "/opt/axon/libaxon_pjrt.so"
    mod = types.ModuleType("antenv.axon_hooks")
    _hook_box = [None]

    def set_axon_ntff_profile_hook(h):
        _hook_box[0] = h

    def get_axon_ntff_profile_hook():
        return _hook_box[0]

    mod.set_axon_ntff_profile_hook = set_axon_ntff_profile_hook
    mod.get_axon_ntff_profile_hook = get_axon_ntff_profile_hook
    _sys.modules["antenv.axon_hooks"] = mod

    try:
        lib = ctypes.CDLL(so_path)
        if not hasattr(lib, "axon_start_nrt_profile"):
            return
        lib.axon_start_nrt_profile.argtypes = [
            ctypes.POINTER(ctypes.c_int64), ctypes.c_size_t]
        lib.axon_start_nrt_profile.restype = ctypes.c_int64
        lib.axon_stop_nrt_profile.argtypes = [ctypes.c_char_p]
        lib.axon_stop_nrt_profile.restype = ctypes.c_int64

        @contextlib.contextmanager
        def _hook(output_dir, device_ids):
            import jax
            jax.devices()
            if device_ids:
                ids = (ctypes.c_int64 * len(device_ids))(*device_ids)
                rc = lib.axon_start_nrt_profile(ids, len(device_ids))
            else:
                rc = lib.axon_start_nrt_profile(None, 0)
            if rc != 0:
                raise RuntimeError(f"axon_start_nrt_profile rc={rc}")
            try:
                yield
            finally:
                n = lib.axon_stop_nrt_profile(str(output_dir).encode())
                print(f"profile: {n} file(s) written to {output_dir}",
                      file=sys.stderr)

        set_axon_ntff_profile_hook(_hook)
    except OSError:
        pass


# --------------------------------------------------------------------------
# program builders
# --------------------------------------------------------------------------

def build_layer_program(kc_in):
    """One BiLSTM direction for BL sequences. kc_in = input dim / 128."""
    bass, tile, bacc, mybir, _ = _mods()
    dt = mybir.dt
    AF = mybir.ActivationFunctionType
    AO = mybir.AluOpType

    nc = bacc.Bacc("TRN2", target_bir_lowering=False, debug=False)
    xT = nc.dram_tensor("xT", [kc_in, 128, NTOK], dt.bfloat16, kind="ExternalInput").ap()
    wih = nc.dram_tensor("wih", [kc_in, 128, 4 * HD], dt.bfloat16, kind="ExternalInput").ap()
    whh = nc.dram_tensor("whh", [2, 128, 4 * HD], dt.bfloat16, kind="ExternalInput").ap()
    bias = nc.dram_tensor("bias", [128, NJ], dt.float32, kind="ExternalInput").ap()
    hout = nc.dram_tensor("hout", [128, 2, T, BL], dt.bfloat16,
                          kind="ExternalOutput").ap()

    NCHUNKS = NTOK // NCH  # 8

    with tile.TileContext(nc) as tc:
        with (
            tc.tile_pool(name="w", bufs=1) as wpool,
            tc.tile_pool(name="big", bufs=1) as big,
            tc.tile_pool(name="gxp", bufs=2) as gxp,
            tc.tile_pool(name="xs", bufs=3) as xs,
            tc.tile_pool(name="st", bufs=1) as st,
            tc.tile_pool(name="ew", bufs=4) as ew,
            tc.tile_pool(name="ps", bufs=2, space="PSUM") as ps,
            tc.tile_pool(name="psg", bufs=3, space="PSUM") as psg,
        ):
            wih_sb = wpool.tile([128, kc_in, 4 * HD], dt.bfloat16)
            whh_sb = wpool.tile([128, 2, 4 * HD], dt.bfloat16)
            bias_sb = wpool.tile([128, NJ], dt.float32)
            for kc in range(kc_in):
                nc.sync.dma_start(wih_sb[:, kc, :], wih[kc])
            for kc in range(2):
                nc.sync.dma_start(whh_sb[:, kc, :], whh[kc])
            nc.sync.dma_start(bias_sb[:], bias[:])
            from concourse.masks import make_identity
            ident = wpool.tile([128, 128], dt.bfloat16)
            make_identity(nc, ident[:])

            hist = big.tile([128, 2, T + 1, BL], dt.bfloat16)
            cst = st.tile([128, 2, BL], dt.float32)
            nc.vector.memset(hist[:, :, 0, :], 0.0)
            nc.vector.memset(cst[:], 0.0)

            # gx compute for chunk n, interleaved into chunk n-1's scan steps
            def gx_block(gxb, xc, j):
                acc = psg.tile([128, NCH], dt.float32, name="acc")
                for kc in range(kc_in):
                    nc.tensor.matmul(
                        acc[:], wih_sb[:, kc, j * 128:(j + 1) * 128],
                        xc[:, kc, :],
                        start=(kc == 0), stop=(kc == kc_in - 1))
                accv = acc[:].rearrange("p (t b) -> p t b", b=BL)
                nc.vector.tensor_scalar_add(gxb[:, j], accv, bias_sb[:, j:j + 1])

            def load_x(n):
                xc = xs.tile([128, kc_in, NCH], dt.bfloat16, name="xc")
                for kc in range(kc_in):
                    nc.sync.dma_start(xc[:, kc, :],
                                      xT[kc, :, n * NCH:(n + 1) * NCH])
                return xc

            def prefill(gxb, tt):
                # identity matmuls drop gx(+bias) for a whole step into PSUM;
                # o-gates go to their own bank so the c-path sigmoid is not
                # gated on them (PSUM deps are bank-granular)
                G1 = ps.tile([128, 6, BL], dt.float32, name="G1")
                nc.tensor.matmul(G1[:], ident[:], gxb[:, 0:6, tt, :],
                                 start=True, stop=False, skip_group_check=True)
                G2 = ps.tile([128, 2, BL], dt.float32, name="G2")
                nc.tensor.matmul(G2[:], ident[:], gxb[:, 6:8, tt, :],
                                 start=True, stop=False, skip_group_check=True)
                return G1, G2

            # chunk 0 gx up front
            xc_cur = load_x(0)
            gx_cur = gxp.tile([128, NJ, TCH, BL], dt.bfloat16, name="gxb")
            for j in range(NJ):
                gx_block(gx_cur, xc_cur, j)
            Gc = prefill(gx_cur, 0)

            # scan; cell (g rows pre-scaled by 2 on host):
            #   w = (sig_g' - 0.5) * sig_i ; c = 2w + sig_f*c ; h = sig_o*tanh(c)
            for n in range(NCHUNKS):
                gx_nxt = None
                if n + 1 < NCHUNKS:
                    xc_nxt = load_x(n + 1)
                    gx_nxt = gxp.tile([128, NJ, TCH, BL], dt.bfloat16,
                                      name="gxb")
                for tt in range(TCH):
                    t = n * TCH + tt
                    G1, G2 = Gc
                    # c-path gates (i,f,g) first
                    for j in range(6):
                        for kc in range(2):
                            nc.tensor.matmul(
                                G1[:, j, :], whh_sb[:, kc, j * 128:(j + 1) * 128],
                                hist[:, kc, t, :], start=False,
                                stop=(j == 5 and kc == 1),
                                skip_group_check=True)
                    # o-gate matmuls + its sigmoid run off the critical path
                    for j in (6, 7):
                        for kc in range(2):
                            nc.tensor.matmul(
                                G2[:, j - 6, :],
                                whh_sb[:, kc, j * 128:(j + 1) * 128],
                                hist[:, kc, t, :], start=False,
                                stop=(j == 7 and kc == 1),
                                skip_group_check=True)
                    # prefill next step's PSUM + spread next chunk's gx matmuls
                    if tt + 1 < TCH:
                        Gc = prefill(gx_cur, tt + 1)
                    elif gx_nxt is not None:
                        Gc = prefill(gx_nxt, 0)
                    if gx_nxt is not None and tt % 4 == 1 and tt // 4 < NJ:
                        gx_block(gx_nxt, xc_nxt, tt // 4)

                    A1 = ew.tile([128, 6, BL], dt.float32, name="A1")
                    nc.scalar.activation(A1[:], G1[:], AF.Sigmoid)
                    A2 = ew.tile([128, 2, BL], dt.float32, name="A2")
                    nc.scalar.activation(A2[:], G2[:], AF.Sigmoid)
                    w = ew.tile([128, 2, BL], dt.float32, name="w")
                    nc.vector.scalar_tensor_tensor(
                        w[:], A1[:, 4:6, :], 0.5, A1[:, 0:2, :],
                        AO.subtract, AO.mult)
                    m1 = ew.tile([128, 2, BL], dt.float32, name="m1")
                    nc.vector.tensor_tensor(m1[:], A1[:, 2:4, :], cst[:],
                                            AO.mult)
                    nc.vector.scalar_tensor_tensor(
                        cst[:], w[:], 2.0, m1[:], AO.mult, AO.add)
                    Tc = ew.tile([128, 2, BL], dt.float32, name="Tc")
                    nc.scalar.activation(Tc[:], cst[:], AF.Tanh)
                    nc.vector.tensor_tensor(hist[:, :, t + 1, :],
                                            A2[:], Tc[:], AO.mult)
                if gx_nxt is not None:
                    gx_cur, xc_cur = gx_nxt, xc_nxt
                # stream finished history out
                t0 = n * TCH
                nc.sync.dma_start(hout[:, :, t0:t0 + TCH, :],
                                  hist[:, :, t0 + 1:t0 + TCH + 1, :])
    nc.compile()
    return nc


def build_crf_program():
    bass, tile, bacc, mybir, _ = _mods()
    dt = mybir.dt
    AF = mybir.ActivationFunctionType
    AO = mybir.AluOpType

    nc = bacc.Bacc("TRN2", target_bir_lowering=False, debug=False)
    hcat = nc.dram_tensor("hcat", [4, 128, NTOK3], dt.bfloat16, kind="ExternalInput").ap()
    linw = nc.dram_tensor("linw", [4, 128, NT], dt.bfloat16, kind="ExternalInput").ap()
    linb = nc.dram_tensor("linb", [NT, 1], dt.float32, kind="ExternalInput").ap()
    etrans = nc.dram_tensor("etrans", [NT, NT], dt.float32, kind="ExternalInput").ap()
    estart = nc.dram_tensor("estart", [NT, 1], dt.float32, kind="ExternalInput").ap()
    eend = nc.dram_tensor("eend", [NT, 1], dt.float32, kind="ExternalInput").ap()
    emitmask = nc.dram_tensor("emitmask", [NT, NTOK3], dt.bfloat16, kind="ExternalInput").ap()
    lastsel = nc.dram_tensor("lastsel", [NT, BC, T], dt.bfloat16, kind="ExternalInput").ap()
    smask = nc.dram_tensor("smask", [1, BC, NREN + 1], dt.float32, kind="ExternalInput").ap()
    part_out = nc.dram_tensor("part_out", [1, BC], dt.float32, kind="ExternalOutput").ap()
    emit_out = nc.dram_tensor("emit_out", [1, 1], dt.float32, kind="ExternalOutput").ap()

    NCHUNKS3 = NTOK3 // NCH  # 4

    with tile.TileContext(nc) as tc:
        with (
            tc.tile_pool(name="w", bufs=1) as wpool,
            tc.tile_pool(name="big", bufs=1) as big,
            tc.tile_pool(name="sm", bufs=4) as sm,
            tc.tile_pool(name="pslg", bufs=2, space="PSUM") as pslg,
            tc.tile_pool(name="ps", bufs=2, space="PSUM") as ps,
        ):
            hc_sb = big.tile([128, 4, NTOK3], dt.bfloat16)
            for kc in range(4):
                nc.sync.dma_start(hc_sb[:, kc, :], hcat[kc])
            lw_sb = wpool.tile([128, 4, NT], dt.bfloat16)
            for kc in range(4):
                nc.sync.dma_start(lw_sb[:, kc, :], linw[kc])
            lb_sb = wpool.tile([NT, 1], dt.float32)
            nc.sync.dma_start(lb_sb[:], linb[:])
            et_sb = wpool.tile([NT, NT], dt.float32)
            nc.sync.dma_start(et_sb[:], etrans[:])
            es_sb = wpool.tile([NT, 1], dt.float32)
            nc.sync.dma_start(es_sb[:], estart[:])
            ee_sb = wpool.tile([NT, 1], dt.float32)
            nc.sync.dma_start(ee_sb[:], eend[:])
            em_sb = big.tile([NT, NTOK3], dt.bfloat16)
            nc.sync.dma_start(em_sb[:], emitmask[:])
            ls_sb = big.tile([NT, BC, T], dt.bfloat16)
            nc.sync.dma_start(ls_sb[:], lastsel[:])
            sm_sb = wpool.tile([1, BC, NREN + 1], dt.float32)
            nc.sync.dma_start(sm_sb[:], smask[:])
            ones_sb = wpool.tile([NT, 1], dt.float32)
            nc.vector.memset(ones_sb[:], 1.0)
            onesrow = wpool.tile([1, NT], dt.float32)
            nc.vector.memset(onesrow[:], 1.0)

            # logits^T [NT, t, b] fp32, and exp(logits)
            logits = big.tile([NT, T, BC], dt.float32)
            for n in range(NCHUNKS3):
                acc = pslg.tile([NT, NCH], dt.float32, name="lg")
                for kc in range(4):
                    nc.tensor.matmul(acc[:], lw_sb[:, kc, :],
                                     hc_sb[:, kc, n * NCH:(n + 1) * NCH],
                                     start=(kc == 0), stop=(kc == 3))
                accv = acc[:].rearrange("p (t b) -> p t b", b=BC)
                nc.vector.tensor_scalar_add(
                    logits[:, n * (NCH // BC):(n + 1) * (NCH // BC), :],
                    accv, lb_sb[:])
            elog = big.tile([NT, T, BC], dt.float32)
            nc.scalar.activation(elog[:], logits[:], AF.Exp)

            # exp-domain forward recursion, two chains of 4 sequences
            NBH = BC // 2
            shist = big.tile([1, BC, NREN + 1], dt.float32)
            nc.vector.memset(shist[:], 1.0)
            ahists = []
            for c in range(2):
                ah = big.tile([NT, NBH, T], dt.float32, name=f"ah{c}")
                nc.vector.tensor_scalar_mul(
                    ah[:, :, 0], elog[:, 0, c * NBH:(c + 1) * NBH], es_sb[:])
                ahists.append(ah)
            for t in range(1, T):
                for c in range(2):
                    ah = ahists[c]
                    bsl = slice(c * NBH, (c + 1) * NBH)
                    y = ps.tile([NT, NBH], dt.float32, name=f"y{c}", bufs=1)
                    nc.tensor.matmul(y[:], et_sb[:], ah[:, :, t - 1],
                                     start=True, stop=True)
                    if t % RENORM_EVERY == 0:
                        r = t // RENORM_EVERY - 1
                        ssum = ps.tile([NT, NBH], dt.float32, name=f"aux{c}", bufs=1)[0:1]
                        nc.tensor.matmul(ssum[:], ones_sb[:], ah[:, :, t - 1],
                                         start=True, stop=True)
                        nc.vector.tensor_copy(shist[:, bsl, r], ssum[:])
                        rinv = sm.tile([1, NBH], dt.float32, name=f"rinv{c}")
                        nc.vector.reciprocal(rinv[:], ssum[:])
                        rb = ps.tile([NT, NBH], dt.float32, name=f"aux{c}", bufs=1)
                        nc.tensor.matmul(rb[:], onesrow[:], rinv[:],
                                         start=True, stop=True)
                        u1 = sm.tile([NT, NBH], dt.float32, name=f"u1{c}")
                        nc.vector.tensor_tensor(u1[:], y[:], elog[:, t, bsl],
                                                AO.mult)
                        nc.vector.tensor_tensor(ah[:, :, t], u1[:], rb[:],
                                                AO.mult)
                    else:
                        nc.vector.tensor_tensor(ah[:, :, t], y[:],
                                                elog[:, t, bsl], AO.mult)

            # partition_b = ln(sum_j a[len_b-1, j] * e_end[j]) + sum_r ln(s_rb)
            alast = sm.tile([NT, BC], dt.float32)
            for c in range(2):
                bsl = slice(c * NBH, (c + 1) * NBH)
                prod = big.tile([NT, NBH, T], dt.float32, name=f"prod{c}")
                nc.vector.tensor_tensor(prod[:], ahists[c][:], ls_sb[:, bsl, :],
                                        AO.mult)
                nc.vector.reduce_sum(alast[:, bsl], prod[:],
                                     axis=mybir.AxisListType.X)
            w2 = sm.tile([NT, BC], dt.float32)
            nc.vector.tensor_scalar_mul(w2[:], alast[:], ee_sb[:])
            fsum = ps.tile([1, BC], dt.float32, name="faux", bufs=1)
            nc.tensor.matmul(fsum[:], ones_sb[:], w2[:], start=True, stop=True)
            pln = sm.tile([1, BC], dt.float32)
            nc.scalar.activation(pln[:], fsum[:], AF.Ln)
            slog = sm.tile([1, BC, NREN + 1], dt.float32)
            nc.scalar.activation(slog[:], shist[:], AF.Ln)
            slogm = sm.tile([1, BC, NREN + 1], dt.float32)
            nc.vector.tensor_tensor(slogm[:], slog[:], sm_sb[:], AO.mult)
            zb = sm.tile([1, BC], dt.float32)
            nc.vector.reduce_sum(zb[:], slogm[:], axis=mybir.AxisListType.X)
            pout = sm.tile([1, BC], dt.float32)
            nc.vector.tensor_tensor(pout[:], pln[:], zb[:], AO.add)
            nc.sync.dma_start(part_out[:], pout[:])

            # emission score total
            eprod = big.tile([NT, T, BC], dt.float32)
            nc.vector.tensor_tensor(
                eprod[:], logits[:],
                em_sb[:].rearrange("p (t b) -> p t b", b=BC), AO.mult)
            erow = sm.tile([NT, 1], dt.float32)
            nc.vector.reduce_sum(erow[:], eprod[:], axis=mybir.AxisListType.XY)
            etot = ps.tile([1, 1], dt.float32, name="faux", bufs=1)
            nc.tensor.matmul(etot[:], ones_sb[:], erow[:], start=True, stop=True)
            eout = sm.tile([1, 1], dt.float32)
            nc.vector.tensor_copy(eout[:], etot[:])
            nc.sync.dma_start(emit_out[:], eout[:])
    nc.compile()
    return nc


# --------------------------------------------------------------------------
# host-side data prep
# --------------------------------------------------------------------------

def _layer_inputs(xin, w_ih, w_hh, b_ih, b_hh):
    """Per-core input dicts for one layer launch.

    xin: [2, B, T, K] fp32 (xin[1] already reversed+masked)
    w_ih: [2, 4HD, K]; w_hh: [2, 4HD, HD]; b_ih, b_hh: [2, 4HD]
    """
    K = xin.shape[-1]
    kc_in = K // 128
    # scale the g-gate rows (post-perm block 3) by 2: tanh(x) = 2*sig(2x)-1
    gscale = np.ones((4 * HD, 1), np.float32)
    gscale[2 * HD:3 * HD] = 2.0
    per_dir = []
    for d in range(2):
        wih_p = w_ih[d][_PERM] * gscale
        whh_p = w_hh[d][_PERM] * gscale
        b_p = (b_ih[d] + b_hh[d])[_PERM] * gscale[:, 0]
        wihT = np.ascontiguousarray(
            wih_p.T.reshape(kc_in, 128, 4 * HD)).astype(BF16)
        whhT = np.ascontiguousarray(
            whh_p.T.reshape(2, 128, 4 * HD)).astype(BF16)
        bs = np.ascontiguousarray(
            b_p.reshape(NJ, 128).T).astype(np.float32)
        per_dir.append((wihT, whhT, bs))
    maps = []
    for core in range(NCORES):
        d, q = divmod(core, 4)
        xc = xin[d, q * BL:(q + 1) * BL]              # [BL, T, K]
        xT = np.ascontiguousarray(
            xc.transpose(2, 1, 0).reshape(kc_in, 128, T * BL)).astype(BF16)
        wihT, whhT, bs = per_dir[d]
        maps.append({"xT": xT, "wih": wihT, "whh": whhT, "bias": bs})
    return maps


def _collect_h(results):
    """per-core 'hout' [128,2,T,BL] bf16 -> h [2, B, T, HD] fp32."""
    h = np.empty((2, B, T, HD), np.float32)
    for core in range(NCORES):
        d, q = divmod(core, 4)
        ho = np.asarray(results[core]["hout"], dtype=np.float32)
        h[d, q * BL:(q + 1) * BL] = ho.transpose(3, 2, 1, 0).reshape(BL, T, HD)
    return h


def _unreverse(h_rev, lens, valid):
    """h_rev[b, s] holds position lens_b-1-s; return h[b, t] (zeros at pad)."""
    t = np.arange(T)
    idx = np.clip(lens[:, None] - 1 - t[None, :], 0, T - 1)
    out = np.take_along_axis(h_rev, idx[:, :, None], axis=1)
    return out * valid[:, :, None]


def kernel(**inputs):
    _, _, _, _, run_bass_kernel_spmd = _mods()
    global LAST_RESULTS
    LAST_RESULTS = []
    trace = bool(int(os.environ.get("KERNEL_TRACE", "0")))
    if trace:
        _install_ntff_shim()

    tokens = np.asarray(inputs["tokens"]).astype(np.int64)
    lens = np.asarray(inputs["lens"]).astype(np.int64)
    labels = np.asarray(inputs["labels"]).astype(np.int64)
    emb = np.asarray(inputs["emb"], dtype=np.float32)
    w_ih = [np.asarray(inputs["w_ih_l0"], np.float32),
            np.asarray(inputs["w_ih_l1"], np.float32)]
    w_hh = [np.asarray(inputs["w_hh_l0"], np.float32),
            np.asarray(inputs["w_hh_l1"], np.float32)]
    b_ih = [np.asarray(inputs["b_ih_l0"], np.float32),
            np.asarray(inputs["b_ih_l1"], np.float32)]
    b_hh = [np.asarray(inputs["b_hh_l0"], np.float32),
            np.asarray(inputs["b_hh_l1"], np.float32)]
    lin_w = np.asarray(inputs["lin_w"], np.float32)
    lin_b = np.asarray(inputs["lin_b"], np.float32)
    trans = np.asarray(inputs["trans"], np.float32)
    start_t = np.asarray(inputs["start_t"], np.float32)
    end_t = np.asarray(inputs["end_t"], np.float32)

    t_ar = np.arange(T)
    valid = (t_ar[None, :] < lens[:, None]).astype(np.float32)
    rev_idx = np.clip(lens[:, None] - 1 - t_ar[None, :], 0, T - 1)

    if "layer0" not in _CACHE:
        _CACHE["layer0"] = build_layer_program(E // 128)
    if "layer1" not in _CACHE:
        _CACHE["layer1"] = build_layer_program(2 * HD // 128)
    if "crf" not in _CACHE:
        _CACHE["crf"] = build_crf_program()

    cores = list(range(NCORES))

    # ---------- launch 1: layer 0 ----------
    x = emb[tokens]
    x_rev = np.take_along_axis(x, rev_idx[:, :, None], axis=1) * valid[:, :, None]
    xin0 = np.stack([x, x_rev])
    res1 = run_bass_kernel_spmd(
        _CACHE["layer0"], _layer_inputs(xin0, w_ih[0], w_hh[0], b_ih[0], b_hh[0]),
        cores, trace=trace)
    LAST_RESULTS.append(res1)
    h0 = _collect_h(res1.results)

    # ---------- launch 2: layer 1 ----------
    h0f = h0[0] * valid[:, :, None]
    h0b = _unreverse(h0[1], lens, valid)
    x1 = np.concatenate([h0f, h0b], axis=-1)
    x1_rev = np.take_along_axis(x1, rev_idx[:, :, None], axis=1) * valid[:, :, None]
    xin1 = np.stack([x1, x1_rev])
    res2 = run_bass_kernel_spmd(
        _CACHE["layer1"], _layer_inputs(xin1, w_ih[1], w_hh[1], b_ih[1], b_hh[1]),
        cores, trace=trace)
    LAST_RESULTS.append(res2)
    h1 = _collect_h(res2.results)

    # ---------- launch 3: logits + CRF ----------
    h1f = h1[0] * valid[:, :, None]
    h1b = _unreverse(h1[1], lens, valid)
    hcat = np.concatenate([h1f, h1b], axis=-1)

    lw = np.ascontiguousarray(lin_w.T.reshape(4, 128, NT)).astype(BF16)
    et = np.exp(trans).astype(np.float32)
    es = np.exp(start_t).astype(np.float32)[:, None]
    ee = np.exp(end_t).astype(np.float32)[:, None]
    lb = np.ascontiguousarray(lin_b.astype(np.float32)[:, None])
    maps = []
    for core in range(NCORES):
        bs = slice(core * BC, (core + 1) * BC)
        hc = hcat[bs]
        hcT = np.ascontiguousarray(
            hc.transpose(2, 1, 0).reshape(4, 128, T * BC)).astype(BF16)
        em = np.zeros((NT, T, BC), np.float32)
        lab = labels[bs]
        for bb in range(BC):
            em[lab[bb], np.arange(T), bb] = valid[bs][bb]
        ls = np.zeros((NT, BC, T), np.float32)
        for bb in range(BC):
            ls[:, bb, lens[bs][bb] - 1] = 1.0
        r_idx = np.arange(NREN + 1)
        smk = (RENORM_EVERY * (r_idx[None] + 1)
               <= (lens[bs] - 1)[:, None]).astype(np.float32)[None]
        maps.append({
            "hcat": hcT, "linw": lw, "linb": lb, "etrans": et,
            "estart": es, "eend": ee,
            "emitmask": np.ascontiguousarray(
                em.reshape(NT, T * BC)).astype(BF16),
            "lastsel": np.ascontiguousarray(ls).astype(BF16),
            "smask": np.ascontiguousarray(smk),
        })
    res3 = run_bass_kernel_spmd(_CACHE["crf"], maps, cores, trace=trace)
    LAST_RESULTS.append(res3)

    partition = np.concatenate(
        [np.asarray(r["part_out"])[0] for r in res3.results])
    emit = float(sum(np.asarray(r["emit_out"])[0, 0] for r in res3.results))

    # host-side numerator terms
    first_tag = labels[:, 0]
    last_tag = np.take_along_axis(labels, (lens - 1)[:, None], axis=1)[:, 0]
    tr_sc = float((trans[labels[:, :-1], labels[:, 1:]] * valid[:, 1:]).sum())
    host_num = float(start_t[first_tag].sum()) + tr_sc + float(end_t[last_tag].sum())

    loss = partition.sum() - emit - host_num
    return np.float32(loss)

